# revision 14
# baseline (speedup 1.0000x reference)
"""Trainium2 Bass kernel for a supervised-contrastive-style loss.

Reference computation (see problem statement):
  - dropout(p=0.5, scale 2, jax key 42) on gathered class-2 rows, concat -> feats [N2, D]
  - fn = feats / max(||feats||, 1e-8);  sim = fn @ fn.T / T
  - denom_i = sum_j exp(sim_ij) * [labs_i == labs_j]
  - loss = -mean(sim_ii - log denom_i)

Strategy:
  * Host: reproduce dropout/concat/normalize (O(N2*D), trivial), then sort rows
    by class.  The label mask becomes block-diagonal, so the device only
    computes same-class row x col tiles (~46% of the full N2^2 work).
  * Device (8 cores, SPMD): each core owns 1/8 of the row-tiles of every class.
    For each (row-tile[128], col-panel[512]) same-class pair: 8 accumulating
    fp32r matmuls (K=1024) into PSUM, then one ScalarE activation
    exp(S * 1/T) with accum_out giving the row-sum of the tile.  A small diag
    pass recomputes each row-tile's diagonal block with identical operands and
    extracts raw sim_ii (bit-identical to the value that went through exp), so
    log(denom_i) - sim_ii cancels structurally on the host.
  * Host: float64 sum of per-tile partials, subtract zero-padding columns
    (each contributes exp(0)), log, mean.
"""

import math

import numpy as np

TEMPERATURE = 0.07
DROP_P = 0.5
EPS = 1e-8
NCORES = 8
KP = 128  # partition size
PANEL = 512  # matmul moving free dim (one PSUM bank of fp32)

_CACHE = {}


# --------------------------------------------------------------------------
# host-side preparation
# --------------------------------------------------------------------------

def _host_prep(features, labels, aug_indices):
    """Mirror the reference's prologue op-for-op on the default jax backend so
    the dropout PRNG bits and fn values match the graded reference exactly."""
    import jax
    import jax.numpy as jnp

    features = jnp.asarray(np.asarray(features))
    labels_np = np.asarray(labels)
    aug_np = np.asarray(aug_indices)
    labels = jnp.asarray(labels_np)
    aug = jnp.asarray(aug_np)

    pert = features[aug]
    keep = jax.random.bernoulli(jax.random.key(42), 1.0 - DROP_P, pert.shape)
    pert = jnp.where(keep, pert * 2.0, jnp.zeros((), dtype=pert.dtype))
    feats = jnp.concatenate([features, pert], axis=0)

    norms = jnp.sqrt(jnp.sum(feats * feats, axis=1, keepdims=True))
    fn = np.asarray(feats / jnp.maximum(norms, EPS)).astype(np.float32)
    labs = np.concatenate([labels_np, labels_np[aug_np]], axis=0)

    perm = np.argsort(labs, kind="stable")
    fn_sorted = np.ascontiguousarray(fn[perm])
    labs_sorted = labs[perm]
    return fn, labs, perm, fn_sorted, labs_sorted


class _Plan:
    """All compile-time structure derived from shapes + class counts."""

    def __init__(self, n2, d, class_counts):
        assert d % KP == 0
        self.n2 = n2
        self.d = d
        self.kt = d // KP                       # k-tiles
        self.counts = list(class_counts)        # rows per class (sorted order)
        self.ncls = len(self.counts)
        # panels per class / per-core row-tiles per class
        self.P = [math.ceil(c / PANEL) for c in self.counts]
        self.R = [math.ceil(math.ceil(c / KP) / NCORES) for c in self.counts]
        self.S = [r * KP for r in self.R]       # per-core row slots per class
        self.row_slots = sum(self.S)            # lhsT columns per core
        self.col_slots = sum(p * PANEL for p in self.P)
        self.colpad = [p * PANEL - c for p, c in zip(self.P, self.counts)]
        self.njobs = sum(p * r for p, r in zip(self.P, self.R))
        self.nrt = sum(self.R)                  # row-tiles per core
        # class offsets
        self.cls_row_off = np.cumsum([0] + self.counts).tolist()   # in sorted rows
        self.slot_off = np.cumsum([0] + self.S).tolist()           # per-core row slots
        self.panel_off = np.cumsum([0] + [p * PANEL for p in self.P]).tolist()

    def job_index(self, c, p, r):
        base = sum(self.P[cc] * self.R[cc] for cc in range(c))
        return base + p * self.R[c] + r

    def rowtile_index(self, c, r):
        return sum(self.R[cc] for cc in range(c)) + r


def _build_host_arrays(plan, fn_sorted):
    """cols tensor (shared by all cores) and per-core lhsT tensors."""
    d, kt = plan.d, plan.kt
    fnT = np.ascontiguousarray(fn_sorted.T)      # [D, N2] f32

    cols = np.zeros((kt, KP, plan.col_slots), dtype=np.float32)
    for c in range(plan.ncls):
        nrows = plan.counts[c]
        src = fnT[:, plan.cls_row_off[c]: plan.cls_row_off[c] + nrows]
        cols[:, :, plan.panel_off[c]: plan.panel_off[c] + nrows] = (
            src.reshape(kt, KP, nrows))

    lhsTs = []
    for core in range(NCORES):
        lt = np.zeros((kt, KP, plan.row_slots), dtype=np.float32)
        for c in range(plan.ncls):
            s0 = core * plan.S[c]
            nreal = min(max(plan.counts[c] - s0, 0), plan.S[c])
            if nreal > 0:
                src = fnT[:, plan.cls_row_off[c] + s0:
                          plan.cls_row_off[c] + s0 + nreal]
                lt[:, :, plan.slot_off[c]: plan.slot_off[c] + nreal] = (
                    src.reshape(kt, KP, nreal))
        lhsTs.append(lt)
    return cols, lhsTs


# --------------------------------------------------------------------------
# bass program
# --------------------------------------------------------------------------

def _build_program(plan, reps=1):
    import contextlib

    import concourse.bacc as bacc
    import concourse.tile as tile
    import concourse.mybir as mybir

    f32 = mybir.dt.float32
    f32r = mybir.dt.float32r
    scale32 = float(np.float32(1.0) / np.float32(TEMPERATURE))

    nc = bacc.Bacc("TRN2", target_bir_lowering=False, debug=False)
    lhsT_d = nc.dram_tensor("lhsT", [plan.kt, KP, plan.row_slots], f32r,
                            kind="ExternalInput")
    cols_d = nc.dram_tensor("cols", [plan.kt, KP, plan.col_slots], f32r,
                            kind="ExternalInput")
    ident_d = nc.dram_tensor("ident", [KP, KP], f32, kind="ExternalInput")
    part_d = nc.dram_tensor("partials", [KP, plan.njobs], f32,
                            kind="ExternalOutput")
    diag_d = nc.dram_tensor("diag", [KP, plan.nrt], f32, kind="ExternalOutput")
    e0_d = nc.dram_tensor("e0", [KP, 1], f32, kind="ExternalOutput")

    with tile.TileContext(nc) as tc:
        with (
            tc.tile_pool(name="persist", bufs=1) as persist,
            tc.tile_pool(name="panels", bufs=4) as panels,
            tc.tile_pool(name="work", bufs=4) as work,
            tc.tile_pool(name="psum", bufs=7, space="PSUM") as psum_main,
            tc.tile_pool(name="psumd", bufs=1, space="PSUM") as psum_diag,
        ):
            lhsT = persist.tile([KP, plan.kt, plan.row_slots], f32r)
            for k in range(plan.kt):
                nc.sync.dma_start(out=lhsT[:, k, :], in_=lhsT_d[k])
            ident = persist.tile([KP, KP], f32)
            nc.sync.dma_start(out=ident, in_=ident_d[:])
            partials = persist.tile([KP, plan.njobs], f32)
            diag = persist.tile([KP, plan.nrt], f32)

            # exp(0) witness (for zero-padding correction on host)
            zt = persist.tile([KP, 1], f32)
            nc.vector.memset(zt, 0.0)
            e0t = persist.tile([KP, 1], f32)
            nc.scalar.activation(out=e0t, in_=zt,
                                 func=mybir.ActivationFunctionType.Exp,
                                 scale=scale32)
            nc.sync.dma_start(out=e0_d[:], in_=e0t)

            # For timing runs, execute the whole compute body `reps` times in
            # a hardware loop; outputs are identical each iteration.
            def emit_body():
                # diag pass: raw sim_ii, bit-identical operands to main pass
                for t in range(plan.nrt):
                    ps = psum_diag.tile([KP, KP], f32)
                    sl = slice(t * KP, (t + 1) * KP)
                    for k in range(plan.kt):
                        nc.tensor.matmul(ps, lhsT[:, k, sl], lhsT[:, k, sl],
                                         start=(k == 0),
                                         stop=(k == plan.kt - 1))
                    tmp = work.tile([KP, KP], f32, tag="dtmp")
                    nc.vector.tensor_mul(tmp, ps, ident)
                    nc.vector.reduce_sum(diag[:, t:t + 1], tmp,
                                         axis=mybir.AxisListType.X)

                # main pass
                for c in range(plan.ncls):
                    for p in range(plan.P[c]):
                        panel = panels.tile([KP, plan.kt, PANEL], f32r)
                        c0 = plan.panel_off[c] + p * PANEL
                        for k in range(plan.kt):
                            nc.sync.dma_start(out=panel[:, k, :],
                                              in_=cols_d[k, :, c0:c0 + PANEL])
                        for r in range(plan.R[c]):
                            ps = psum_main.tile([KP, PANEL], f32)
                            rsl = slice(plan.slot_off[c] + r * KP,
                                        plan.slot_off[c] + (r + 1) * KP)
                            for k in range(plan.kt):
                                nc.tensor.matmul(ps, lhsT[:, k, rsl],
                                                 panel[:, k, :],
                                                 start=(k == 0),
                                                 stop=(k == plan.kt - 1))
                            e = work.tile([KP, PANEL], f32, tag="etile")
                            j = plan.job_index(c, p, r)
                            nc.scalar.activation(
                                out=e, in_=ps,
                                func=mybir.ActivationFunctionType.Exp,
                                scale=scale32,
                                accum_out=partials[:, j:j + 1])

            if reps > 1:
                with tc.For_i(0, reps, 1):
                    emit_body()
            else:
                emit_body()

            nc.sync.dma_start(out=part_d[:], in_=partials)
            nc.sync.dma_start(out=diag_d[:], in_=diag)
    nc.compile()
    return nc


# --------------------------------------------------------------------------
# numpy simulation of the device outputs (for logic validation)
# --------------------------------------------------------------------------

def _simulate_device(plan, cols, lhsTs):
    scale32 = np.float32(1.0) / np.float32(TEMPERATURE)
    results = []
    kt = plan.kt
    colsf = cols.reshape(kt * KP, plan.col_slots)
    for core in range(NCORES):
        lt = lhsTs[core].reshape(kt * KP, plan.row_slots)
        partials = np.zeros((KP, plan.njobs), dtype=np.float32)
        diag = np.zeros((KP, plan.nrt), dtype=np.float32)
        for c in range(plan.ncls):
            for r in range(plan.R[c]):
                rsl = slice(plan.slot_off[c] + r * KP,
                            plan.slot_off[c] + (r + 1) * KP)
                blk = (lt[:, rsl].T @ lt[:, rsl]).astype(np.float32)
                diag[:, plan.rowtile_index(c, r)] = np.diag(blk)
                for p in range(plan.P[c]):
                    c0 = plan.panel_off[c] + p * PANEL
                    s = (lt[:, rsl].T @ colsf[:, c0:c0 + PANEL]).astype(np.float32)
                    e = np.exp((s * scale32).astype(np.float32),
                               dtype=np.float32)
                    partials[:, plan.job_index(c, p, r)] = e.sum(
                        axis=1, dtype=np.float32)
        results.append({"partials": partials, "diag": diag,
                        "e0": np.ones((KP, 1), dtype=np.float32)})
    return results


# --------------------------------------------------------------------------
# host-side finish
# --------------------------------------------------------------------------

def _finish(plan, results):
    """Combine per-core device outputs into the scalar loss (float64 host math).

    For a real row (class c, class-row index g = core*S_c + r*128 + i):
      denom_g = sum_p partials[i, job(c,p,r)] - colpad_c * exp_dev(0)
      x_g     = f32(diag[i, rowtile(c,r)] * f32(1/T))   (matches ACT's scaling)
      loss_g  = log(denom_g) - x_g
    """
    scale32 = np.float32(1.0) / np.float32(TEMPERATURE)
    total = 0.0
    nrows = 0
    for core in range(NCORES):
        partials = results[core]["partials"].astype(np.float64)
        diag = results[core]["diag"]
        e0 = float(results[core]["e0"][0, 0])
        for c in range(plan.ncls):
            s0 = core * plan.S[c]
            nreal = min(max(plan.counts[c] - s0, 0), plan.S[c])
            if nreal <= 0:
                continue
            pad_corr = plan.colpad[c] * e0
            for r in range(plan.R[c]):
                lo = r * KP
                if lo >= nreal:
                    break
                m = min(KP, nreal - lo)  # real partitions in this tile
                jidx = [plan.job_index(c, p, r) for p in range(plan.P[c])]
                denom = partials[:m, jidx].sum(axis=1) - pad_corr
                x = (diag[:m, plan.rowtile_index(c, r)].astype(np.float32)
                     * scale32).astype(np.float32).astype(np.float64)
                total += float(np.sum(np.log(denom) - x))
                nrows += m
    assert nrows == plan.n2, (nrows, plan.n2)
    return np.float32(total / nrows)


# --------------------------------------------------------------------------
# entry point
# --------------------------------------------------------------------------

def _get_compiled(plan, reps=1):
    key = (plan.n2, plan.d, tuple(plan.counts), reps)
    if key not in _CACHE:
        _CACHE[key] = _build_program(plan, reps=reps)
    return _CACHE[key]


def _prepare(inputs):
    features = np.asarray(inputs["features"])
    labels = np.asarray(inputs["labels"])
    aug_indices = np.asarray(inputs["aug_indices"])

    fn, labs, perm, fn_sorted, labs_sorted = _host_prep(
        features, labels, aug_indices)
    n2, d = fn_sorted.shape
    classes, counts = np.unique(labs_sorted, return_counts=True)
    plan = _Plan(n2, d, counts.tolist())
    cols, lhsTs = _build_host_arrays(plan, fn_sorted)
    ident = np.eye(KP, dtype=np.float32)
    in_maps = [{"lhsT": lhsTs[core], "cols": cols, "ident": ident}
               for core in range(NCORES)]
    return plan, cols, lhsTs, in_maps


def kernel(simulate=False, **inputs):
    plan, cols, lhsTs, in_maps = _prepare(inputs)

    if simulate:
        results = _simulate_device(plan, cols, lhsTs)
    else:
        from concourse.bass_utils import run_bass_kernel_spmd

        nc = _get_compiled(plan)
        results = run_bass_kernel_spmd(nc, in_maps,
                                       core_ids=list(range(NCORES))).results

    return np.asarray(_finish(plan, results), dtype=np.float32)


# --------------------------------------------------------------------------
# timing harness (mirrors bass2jax.run_bass_via_pjrt's multi-core path but
# keeps the big inputs device-resident so repeated calls time the NEFF)
# --------------------------------------------------------------------------

def _make_sharded(nc, n_cores):
    import jax
    import concourse.mybir as mybir
    from jax.sharding import Mesh, PartitionSpec
    from jax.experimental.shard_map import shard_map
    from concourse.bass2jax import (_bass_exec_p, install_neuronx_cc_hook,
                                    partition_id_tensor)

    install_neuronx_cc_hook()
    partition_name = (nc.partition_id_tensor.name
                      if nc.partition_id_tensor else None)
    in_names, out_names, out_avals, zero_outs = [], [], [], []
    for alloc in nc.m.functions[0].allocations:
        if not isinstance(alloc, mybir.MemoryLocationSet):
            continue
        name = alloc.memorylocations[0].name
        if alloc.kind == "ExternalInput":
            if name != partition_name:
                in_names.append(name)
        elif alloc.kind == "ExternalOutput":
            out_names.append(name)
            shape = tuple(alloc.tensor_shape)
            dtype = mybir.dt.np(alloc.dtype)
            out_avals.append(jax.core.ShapedArray(shape, dtype))
            zero_outs.append(np.zeros(shape, dtype))
    n_params = len(in_names)
    all_names = in_names + out_names
    if partition_name is not None:
        all_names.append(partition_name)

    def _body(*args):
        operands = list(args)
        if partition_name is not None:
            operands.append(partition_id_tensor())
        outs = _bass_exec_p.bind(
            *operands,
            out_avals=tuple(out_avals),
            in_names=tuple(all_names),
            out_names=tuple(out_names),
            lowering_input_output_aliases=(),
            sim_require_finite=True,
            sim_require_nnan=True,
            nc=nc,
        )
        return tuple(outs)

    devices = jax.devices()[:n_cores]
    mesh = Mesh(np.asarray(devices), ("core",))
    in_specs = (PartitionSpec("core"),) * (n_params + len(out_names))
    out_specs = (PartitionSpec("core"),) * len(out_names)
    donate = tuple(range(n_params, n_params + len(out_names)))
    sharded = jax.jit(
        shard_map(_body, mesh=mesh, in_specs=in_specs, out_specs=out_specs,
                  check_rep=False),
        donate_argnums=donate, keep_unused=True)
    return sharded, in_names, out_names, out_avals, zero_outs, mesh


def _make_runner(nc, in_maps):
    import jax
    from jax.sharding import NamedSharding, PartitionSpec

    sharded, in_names, out_names, out_avals, zero_outs, mesh = _make_sharded(
        nc, NCORES)
    concat_in = [np.concatenate([in_maps[c][n] for c in range(NCORES)], axis=0)
                 for n in in_names]
    sharding = NamedSharding(mesh, PartitionSpec("core"))
    dev_in = [jax.device_put(a, sharding) for a in concat_in]

    def run():
        zs = [jax.device_put(
            np.zeros((NCORES * z.shape[0], *z.shape[1:]), z.dtype), sharding)
            for z in zero_outs]
        jax.block_until_ready(zs)
        import time
        t0 = time.perf_counter()
        out = sharded(*dev_in, *zs)
        jax.block_until_ready(out)
        return time.perf_counter() - t0

    run()  # warmup (compile + first exec)
    return run


def benchmark(loop_reps=129, pairs=10, **inputs):
    """Per-iteration kernel time, cancelling the ~100ms axon dispatch floor:
    interleave timings of a 1-rep NEFF and a `loop_reps`-rep NEFF (HW loop)
    and difference the minima."""
    plan, cols, lhsTs, in_maps = _prepare(inputs)
    run1 = _make_runner(_get_compiled(plan, reps=1), in_maps)
    runR = _make_runner(_get_compiled(plan, reps=loop_reps), in_maps)

    t1s, tRs = [], []
    for _ in range(pairs):
        t1s.append(run1())
        tRs.append(runR())
    m1, mR = min(t1s), min(tRs)
    per_iter = (mR - m1) / (loop_reps - 1)
    print(f"  [bench] min T(1)={m1*1e3:.2f}ms  min T({loop_reps})={mR*1e3:.2f}ms")
    return per_iter * 1e9
